# revision 6
# baseline (speedup 1.0000x reference)
"""3x3 median blur (replicate padding) on Trainium2, 8-core data parallel.

v3: variable-size tiles [4,8,8,4] — fewer instructions than v2 (72 vs 108
DVE ops) while keeping the ramp/tail DMAs small.

Problem: noised_image [32,3,512,512] f32 -> median-blurred; cover_image
passthrough.

- Shard batch across 8 NeuronCores: 4 images (12 channel-planes) per core.
- Host-side: cast f32 -> f16 (median commutes with monotone quantization;
  error bounded by the f16 quantization step ~1e-3 << the 2e-2 gate),
  edge-pad each plane to 514x514, column-interleave plane PAIRS:
  T[r, 2c] = A[r, c], T[r, 2c+1] = B[r, c]  -> 6 pair-strips of [514, 1028].
- Device: 4 tiles per core: tile0 = pair-strip 0 (R=4 rows/partition),
  tile1 = strips 1+2 (R=8: partitions 0-63 strip 1, 64-127 strip 2),
  tile2 = strips 3+4 (R=8), tile3 = strip 5 (R=4). 18 DVE f16
  tensor_tensor ops per tile (2x_1p perf mode ~2 elem/lane/cycle) using a
  6-buffer no-in-place rotation schedule:
    vertical sort3:  pmn,pmx -> lo,hi,tt,mid
    horizontal:      e1,A | f1,C | qmn,qmx,u,B | fmn,fmx,v,out
- Raw Bass program (explicit semaphores), double-buffered input/output
  tiles, DMA on the sync (SP/HWDGE) engine overlapping compute. Output is
  the interleaved f16 pair; host de-interleaves and casts back to f32.
"""
import sys
sys.path.insert(0, '/opt/trn_rl_repo')
from contextlib import ExitStack
import numpy as np

import concourse.bass as bass
import concourse.mybir as mybir
import bass_rust
from concourse import bass_utils

F16 = mybir.dt.float16
MIN = mybir.AluOpType.min
MAX = mybir.AluOpType.max

N_CORES = 8
N_CH = 12          # channel-planes per core (4 images x 3 channels)
N_STR = 6          # interleaved pair-strips per core
H = W = 512
HP = 514           # host-padded plane height
WI = 2 * 514       # interleaved padded width (f16 elems)
WO = 2 * 512       # interleaved output width
RMAX = 8
# (first pair-strip, strips spanned, rows per partition)
TILES = [(0, 1, 4), (1, 2, 8), (3, 2, 8), (5, 1, 4)]


def _mk_ap(base, dims, offset):
    c = base.copy()
    c.ap = bass_rust.VecI64Pair(dims)
    c.offset = offset
    return c


def _build_nc(reps=1, use_gpsimd=False):
    nc = bass.Bass("TRN2")
    x = nc.dram_tensor("x", [N_STR, HP, WI], F16, kind="ExternalInput")
    y = nc.dram_tensor("y", [N_STR, W, WO], F16, kind="ExternalOutput")
    with ExitStack() as ctx:
        xs = [ctx.enter_context(nc.sbuf_tensor(f"xs{i}", [128, RMAX + 2, WI], F16)) for i in range(2)]
        ov = [ctx.enter_context(nc.sbuf_tensor(f"ov{i}", [128, RMAX, WO], F16)) for i in range(2)]
        # 6 rotating work buffers
        P = ctx.enter_context(nc.sbuf_tensor("P", [128, RMAX, WI], F16))
        Q = ctx.enter_context(nc.sbuf_tensor("Q", [128, RMAX, WI], F16))
        L = ctx.enter_context(nc.sbuf_tensor("L", [128, RMAX, WI], F16))
        Hb = ctx.enter_context(nc.sbuf_tensor("Hb", [128, RMAX, WI], F16))
        T = ctx.enter_context(nc.sbuf_tensor("T", [128, RMAX, WI], F16))
        M = ctx.enter_context(nc.sbuf_tensor("M", [128, RMAX, WI], F16))
        sem_in = ctx.enter_context(nc.semaphore())
        sem_out = ctx.enter_context(nc.semaphore())
        sem_dve = ctx.enter_context(nc.semaphore())

        block = ctx.enter_context(nc.Block())
        n_tiles = len(TILES) * reps

        def tile(i):
            return TILES[i % len(TILES)]

        def dma_in(sync, i):
            """Input DMA(s) for tile i; increments sem_in by 32 total."""
            s0, ns, r = tile(i)
            X = xs[i % 2]
            if ns == 1:
                src = _mk_ap(x[0], [[r * WI, 128], [WI, r + 2], [1, WI]], s0 * HP * WI)
                sync.dma_start(X[:, 0:r + 2, :], src).then_inc(sem_in, 32)
            else:
                # two 3-dim-AP DMAs, one per 64-partition half (4-dim APs
                # mis-transfer on real HWDGE even though CoreSim walks them)
                for h in range(2):
                    src = _mk_ap(x[0], [[r * WI, 64], [WI, r + 2], [1, WI]],
                                 (s0 + h) * HP * WI)
                    sync.dma_start(X[64 * h:64 * (h + 1), 0:r + 2, :], src).then_inc(sem_in, 16)

        def dma_out(sync, i):
            """Output DMA(s) for tile i; increments sem_out by 32 total."""
            s0, ns, r = tile(i)
            O = ov[i % 2]
            if ns == 1:
                dst = _mk_ap(y[0], [[r * WO, 128], [WO, r], [1, WO]], s0 * W * WO)
                sync.dma_start(dst, O[:, 0:r, :]).then_inc(sem_out, 32)
            else:
                for h in range(2):
                    dst = _mk_ap(y[0], [[r * WO, 64], [WO, r], [1, WO]],
                                 (s0 + h) * W * WO)
                    sync.dma_start(dst, O[64 * h:64 * (h + 1), 0:r, :]).then_inc(sem_out, 16)

        @block.sync
        def _(sync):
            for i in range(n_tiles):
                if i >= 2:
                    # tile i-2's last read of xs[(i-2)%2] is op 5 (tt)
                    sync.wait_ge(sem_dve, 2 * (i - 2) + 1)
                dma_in(sync, i)
                if i >= 1:
                    oi = i - 1
                    sync.wait_ge(sem_dve, 2 * (oi + 1))
                    dma_out(sync, oi)
            oi = n_tiles - 1
            sync.wait_ge(sem_dve, 2 * (oi + 1))
            dma_out(sync, oi)

        @block.vector
        def _(vector):
            for i in range(n_tiles):
                r = tile(i)[2]
                X = xs[i % 2]
                vector.wait_ge(sem_in, 32 * (i + 1))
                # vertical sort3 per column: windows j..j+2 for j=0..r-1
                vector.tensor_tensor(P[:, 0:r, :], X[:, 0:r, :], X[:, 1:r + 1, :], MIN)        # pmn
                vector.tensor_tensor(Q[:, 0:r, :], X[:, 0:r, :], X[:, 1:r + 1, :], MAX)        # pmx
                vector.tensor_tensor(L[:, 0:r, :], P[:, 0:r, :], X[:, 2:r + 2, :], MIN)        # lo
                vector.tensor_tensor(Hb[:, 0:r, :], Q[:, 0:r, :], X[:, 2:r + 2, :], MAX)       # hi
                t = vector.tensor_tensor(T[:, 0:r, :], Q[:, 0:r, :], X[:, 2:r + 2, :], MIN)    # tt (last X read)
                t.then_inc(sem_dve, 1)
                vector.tensor_tensor(M[:, 0:r, :], P[:, 0:r, :], T[:, 0:r, :], MAX)            # mid
                # horizontal stage; plane-tap +k is element offset +2k
                vector.tensor_tensor(P[:, 0:r, 0:WO], L[:, 0:r, 0:WO], L[:, 0:r, 4:WI], MAX)       # e1
                vector.tensor_tensor(Q[:, 0:r, 0:WO], P[:, 0:r, 0:WO], L[:, 0:r, 2:WO + 2], MAX)   # A
                vector.tensor_tensor(L[:, 0:r, 0:WO], Hb[:, 0:r, 0:WO], Hb[:, 0:r, 4:WI], MIN)     # f1
                vector.tensor_tensor(T[:, 0:r, 0:WO], L[:, 0:r, 0:WO], Hb[:, 0:r, 2:WO + 2], MIN)  # C
                vector.tensor_tensor(P[:, 0:r, 0:WO], M[:, 0:r, 0:WO], M[:, 0:r, 4:WI], MIN)       # qmn
                vector.tensor_tensor(Hb[:, 0:r, 0:WO], M[:, 0:r, 0:WO], M[:, 0:r, 4:WI], MAX)      # qmx
                vector.tensor_tensor(L[:, 0:r, 0:WO], Hb[:, 0:r, 0:WO], M[:, 0:r, 2:WO + 2], MIN)  # u
                vector.tensor_tensor(M[:, 0:r, 0:WO], P[:, 0:r, 0:WO], L[:, 0:r, 0:WO], MAX)       # B
                vector.tensor_tensor(P[:, 0:r, 0:WO], Q[:, 0:r, 0:WO], M[:, 0:r, 0:WO], MIN)       # fmn
                vector.tensor_tensor(L[:, 0:r, 0:WO], Q[:, 0:r, 0:WO], M[:, 0:r, 0:WO], MAX)       # fmx
                vector.tensor_tensor(Hb[:, 0:r, 0:WO], L[:, 0:r, 0:WO], T[:, 0:r, 0:WO], MIN)      # v
                if i >= 2:
                    vector.wait_ge(sem_out, 32 * (i - 1))
                t = vector.tensor_tensor(ov[i % 2][:, 0:r, :], P[:, 0:r, 0:WO], Hb[:, 0:r, 0:WO], MAX)
                t.then_inc(sem_dve, 1)
    return nc


_NC_CACHE = {}


def _get_nc(use_gpsimd=False):
    key = use_gpsimd
    if key not in _NC_CACHE:
        _NC_CACHE[key] = _build_nc(use_gpsimd=use_gpsimd)
    return _NC_CACHE[key]


def make_in_maps(noised_image):
    """f32 [32,3,512,512] -> per-core {'x': [6, 514, 1028] f16 interleaved}."""
    per = noised_image.shape[0] // N_CORES
    in_maps = []
    for c in range(N_CORES):
        shard = noised_image[c * per:(c + 1) * per].reshape(N_CH, H, W)
        shard16 = shard.astype(np.float16)
        padded = np.pad(shard16, ((0, 0), (1, 1), (1, 1)), mode='edge')
        inter = np.empty((N_STR, HP, WI), dtype=np.float16)
        inter[:, :, 0::2] = padded[0::2]
        inter[:, :, 1::2] = padded[1::2]
        in_maps.append({"x": np.ascontiguousarray(inter)})
    return in_maps


def kernel(noised_image, cover_image):
    noised_image = np.ascontiguousarray(noised_image, dtype=np.float32)
    nc = _get_nc()
    in_maps = make_in_maps(noised_image)
    res = bass_utils.run_bass_kernel_spmd(nc, in_maps, core_ids=list(range(N_CORES)))
    blurred = np.empty((N_CORES, N_CH, H, W), dtype=np.float16)
    for c, r in enumerate(res.results):
        yc = np.asarray(r["y"]).reshape(N_STR, H, W, 2)
        blurred[c, 0::2] = yc[..., 0]
        blurred[c, 1::2] = yc[..., 1]
    return (blurred.reshape(noised_image.shape).astype(np.float32), cover_image)


# revision 12
# speedup vs baseline: 1.2191x; 1.2191x over previous
"""3x3 median blur (replicate padding) on Trainium2, 8-core data parallel.

v3: variable-size tiles [4,8,8,4] — fewer instructions than v2 (72 vs 108
DVE ops) while keeping the ramp/tail DMAs small.

Problem: noised_image [32,3,512,512] f32 -> median-blurred; cover_image
passthrough.

- Shard batch across 8 NeuronCores: 4 images (12 channel-planes) per core.
- Host-side: cast f32 -> f16 (median commutes with monotone quantization;
  error bounded by the f16 quantization step ~1e-3 << the 2e-2 gate),
  edge-pad each plane to 514x514, column-interleave plane PAIRS:
  T[r, 2c] = A[r, c], T[r, 2c+1] = B[r, c]  -> 6 pair-strips of [514, 1028].
- Device: 4 tiles per core: tile0 = pair-strip 0 (R=4 rows/partition),
  tile1 = strips 1+2 (R=8: partitions 0-63 strip 1, 64-127 strip 2),
  tile2 = strips 3+4 (R=8), tile3 = strip 5 (R=4). 18 DVE f16
  tensor_tensor ops per tile (2x_1p perf mode ~2 elem/lane/cycle) using a
  6-buffer no-in-place rotation schedule:
    vertical sort3:  pmn,pmx -> lo,hi,tt,mid
    horizontal:      e1,A | f1,C | qmn,qmx,u,B | fmn,fmx,v,out
- Raw Bass program (explicit semaphores), double-buffered input/output
  tiles, DMA on the sync (SP/HWDGE) engine overlapping compute. Output is
  the interleaved f16 pair; host de-interleaves and casts back to f32.
"""
import sys
sys.path.insert(0, '/opt/trn_rl_repo')
from contextlib import ExitStack
import numpy as np

import concourse.bass as bass
import concourse.mybir as mybir
import bass_rust
from concourse import bass_utils

F16 = mybir.dt.float16
MIN = mybir.AluOpType.min
MAX = mybir.AluOpType.max

N_CORES = 8
N_CH = 12          # channel-planes per core (4 images x 3 channels)
N_STR = 6          # interleaved pair-strips per core
H = W = 512
HP = 514           # host-padded plane height
WI = 2 * 514       # interleaved padded width (f16 elems)
WO = 2 * 512       # interleaved output width
RMAX = 8
# (first pair-strip, strips spanned, rows per partition)
TILES = [(0, 1, 4), (1, 2, 8), (3, 2, 8), (5, 1, 4)]


def _mk_ap(base, dims, offset):
    c = base.copy()
    c.ap = bass_rust.VecI64Pair(dims)
    c.offset = offset
    return c


def _build_nc(reps=1, use_gpsimd=False):
    nc = bass.Bass("TRN2")
    x = nc.dram_tensor("x", [N_STR, HP, WI], F16, kind="ExternalInput")
    y = nc.dram_tensor("y", [N_STR, W, WO], F16, kind="ExternalOutput")
    with ExitStack() as ctx:
        xs = [ctx.enter_context(nc.sbuf_tensor(f"xs{i}", [128, RMAX + 2, WI], F16)) for i in range(2)]
        ov = [ctx.enter_context(nc.sbuf_tensor(f"ov{i}", [128, RMAX, WO], F16)) for i in range(2)]
        # 6 rotating work buffers
        P = ctx.enter_context(nc.sbuf_tensor("P", [128, RMAX, WI], F16))
        Q = ctx.enter_context(nc.sbuf_tensor("Q", [128, RMAX, WI], F16))
        L = ctx.enter_context(nc.sbuf_tensor("L", [128, RMAX, WI], F16))
        Hb = ctx.enter_context(nc.sbuf_tensor("Hb", [128, RMAX, WI], F16))
        T = ctx.enter_context(nc.sbuf_tensor("T", [128, RMAX, WI], F16))
        M = ctx.enter_context(nc.sbuf_tensor("M", [128, RMAX, WI], F16))
        sem_in = ctx.enter_context(nc.semaphore())
        sem_out = ctx.enter_context(nc.semaphore())
        sem_dve = ctx.enter_context(nc.semaphore())
        # ramp semaphores: each is incremented by exactly ONE DMA (tile 0's
        # part a / part b), so a >=16 wait is sound regardless of SDMA engine
        # skew. (A shared counting sem with partial thresholds is racy: fast
        # engines' later-DMA increments can satisfy the threshold while slow
        # engines' earlier chunks are still in flight.)
        sem_ra = ctx.enter_context(nc.semaphore())
        sem_rb = ctx.enter_context(nc.semaphore())
        # tail semaphore: incremented once, by the globally-last tile's
        # first half-output op (DVE increments are program-ordered, so a
        # partial threshold is sound here, but a dedicated sem keeps the
        # sem_dve accounting untouched)
        sem_tail = ctx.enter_context(nc.semaphore())

        block = ctx.enter_context(nc.Block())
        n_tiles = len(TILES) * reps

        def tile(i):
            return TILES[i % len(TILES)]

        def dma_in(sync, i):
            """Input DMA(s) for tile i; increments sem_in by 32 total."""
            s0, ns, r = tile(i)
            X = xs[i % 2]
            if ns == 1:
                if i == 0:
                    # ramp: rows 0..r land first (ops 1-2 read only rows
                    # 0:r+1), the final row follows; DVE starts ~0.8us
                    # earlier. Dedicated sems sem_ra/sem_rb (see above);
                    # tile 0 contributes nothing to sem_in.
                    a = _mk_ap(x[0], [[r * WI, 128], [WI, r + 1], [1, WI]], s0 * HP * WI)
                    sync.dma_start(X[:, 0:r + 1, :], a).then_inc(sem_ra, 16)
                    b = _mk_ap(x[0], [[r * WI, 128], [WI, 1], [1, WI]],
                               s0 * HP * WI + (r + 1) * WI)
                    sync.dma_start(X[:, r + 1:r + 2, :], b).then_inc(sem_rb, 16)
                    return
                src = _mk_ap(x[0], [[r * WI, 128], [WI, r + 2], [1, WI]], s0 * HP * WI)
                sync.dma_start(X[:, 0:r + 2, :], src).then_inc(sem_in, 32)
            else:
                # two 3-dim-AP DMAs, one per 64-partition half (4-dim APs
                # mis-transfer on real HWDGE even though CoreSim walks them)
                for h in range(2):
                    src = _mk_ap(x[0], [[r * WI, 64], [WI, r + 2], [1, WI]],
                                 (s0 + h) * HP * WI)
                    sync.dma_start(X[64 * h:64 * (h + 1), 0:r + 2, :], src).then_inc(sem_in, 16)

        def dma_out(sync, i):
            """Output DMA(s) for tile i; increments sem_out by 32 total."""
            s0, ns, r = tile(i)
            O = ov[i % 2]
            if ns == 1:
                dst = _mk_ap(y[0], [[r * WO, 128], [WO, r], [1, WO]], s0 * W * WO)
                sync.dma_start(dst, O[:, 0:r, :]).then_inc(sem_out, 32)
            else:
                for h in range(2):
                    dst = _mk_ap(y[0], [[r * WO, 64], [WO, r], [1, WO]],
                                 (s0 + h) * W * WO)
                    sync.dma_start(dst, O[64 * h:64 * (h + 1), 0:r, :]).then_inc(sem_out, 16)

        @block.sync
        def _(sync):
            for i in range(n_tiles):
                if i >= 2:
                    # tile i-2's last read of xs[(i-2)%2] is op 5 (tt)
                    sync.wait_ge(sem_dve, 2 * (i - 2) + 1)
                dma_in(sync, i)
                if i >= 1:
                    oi = i - 1
                    sync.wait_ge(sem_dve, 2 * (oi + 1))
                    dma_out(sync, oi)
            # final tile's output goes out in two halves: the first half
            # streams while the DVE computes the second (tile(n-1) is
            # single-strip, so both halves are full-rate 128-partition DMAs)
            oi = n_tiles - 1
            s0, _, ro = tile(oi)
            hh = ro // 2
            sync.wait_ge(sem_tail, 1)
            d1 = _mk_ap(y[0], [[ro * WO, 128], [WO, hh], [1, WO]], s0 * W * WO)
            sync.dma_start(d1, ov[oi % 2][:, 0:hh, :]).then_inc(sem_out, 16)
            sync.wait_ge(sem_dve, 2 * (oi + 1))
            d2 = _mk_ap(y[0], [[ro * WO, 128], [WO, ro - hh], [1, WO]],
                        s0 * W * WO + hh * WO)
            sync.dma_start(d2, ov[oi % 2][:, hh:ro, :]).then_inc(sem_out, 16)

        @block.vector
        def _(vector):
            for i in range(n_tiles):
                r = tile(i)[2]
                X = xs[i % 2]
                # tile 0 increments sem_ra/sem_rb instead of sem_in, so tiles
                # i>=1 wait for 32*i (tiles 1..i each contribute 32)
                vector.wait_ge(sem_ra if i == 0 else sem_in,
                               16 if i == 0 else 32 * i)
                # vertical sort3 per column: windows j..j+2 for j=0..r-1
                vector.tensor_tensor(P[:, 0:r, :], X[:, 0:r, :], X[:, 1:r + 1, :], MIN)        # pmn
                vector.tensor_tensor(Q[:, 0:r, :], X[:, 0:r, :], X[:, 1:r + 1, :], MAX)        # pmx
                if i == 0:
                    vector.wait_ge(sem_rb, 16)
                vector.tensor_tensor(L[:, 0:r, :], P[:, 0:r, :], X[:, 2:r + 2, :], MIN)        # lo
                vector.tensor_tensor(Hb[:, 0:r, :], Q[:, 0:r, :], X[:, 2:r + 2, :], MAX)       # hi
                t = vector.tensor_tensor(T[:, 0:r, :], Q[:, 0:r, :], X[:, 2:r + 2, :], MIN)    # tt (last X read)
                t.then_inc(sem_dve, 1)
                vector.tensor_tensor(M[:, 0:r, :], P[:, 0:r, :], T[:, 0:r, :], MAX)            # mid
                # horizontal stage; plane-tap +k is element offset +2k
                vector.tensor_tensor(P[:, 0:r, 0:WO], L[:, 0:r, 0:WO], L[:, 0:r, 4:WI], MAX)       # e1
                vector.tensor_tensor(Q[:, 0:r, 0:WO], P[:, 0:r, 0:WO], L[:, 0:r, 2:WO + 2], MAX)   # A
                vector.tensor_tensor(L[:, 0:r, 0:WO], Hb[:, 0:r, 0:WO], Hb[:, 0:r, 4:WI], MIN)     # f1
                vector.tensor_tensor(T[:, 0:r, 0:WO], L[:, 0:r, 0:WO], Hb[:, 0:r, 2:WO + 2], MIN)  # C
                vector.tensor_tensor(P[:, 0:r, 0:WO], M[:, 0:r, 0:WO], M[:, 0:r, 4:WI], MIN)       # qmn
                vector.tensor_tensor(Hb[:, 0:r, 0:WO], M[:, 0:r, 0:WO], M[:, 0:r, 4:WI], MAX)      # qmx
                vector.tensor_tensor(L[:, 0:r, 0:WO], Hb[:, 0:r, 0:WO], M[:, 0:r, 2:WO + 2], MIN)  # u
                vector.tensor_tensor(M[:, 0:r, 0:WO], P[:, 0:r, 0:WO], L[:, 0:r, 0:WO], MAX)       # B
                vector.tensor_tensor(P[:, 0:r, 0:WO], Q[:, 0:r, 0:WO], M[:, 0:r, 0:WO], MIN)       # fmn
                vector.tensor_tensor(L[:, 0:r, 0:WO], Q[:, 0:r, 0:WO], M[:, 0:r, 0:WO], MAX)       # fmx
                vector.tensor_tensor(Hb[:, 0:r, 0:WO], L[:, 0:r, 0:WO], T[:, 0:r, 0:WO], MIN)      # v
                if i >= 2:
                    vector.wait_ge(sem_out, 32 * (i - 1))
                if i == n_tiles - 1:
                    # split the final output op so the first half's out-DMA
                    # overlaps the second half's compute
                    hh = r // 2
                    t = vector.tensor_tensor(ov[i % 2][:, 0:hh, :],
                                             P[:, 0:hh, 0:WO], Hb[:, 0:hh, 0:WO], MAX)
                    t.then_inc(sem_tail, 1)
                    t = vector.tensor_tensor(ov[i % 2][:, hh:r, :],
                                             P[:, hh:r, 0:WO], Hb[:, hh:r, 0:WO], MAX)
                    t.then_inc(sem_dve, 1)
                else:
                    t = vector.tensor_tensor(ov[i % 2][:, 0:r, :], P[:, 0:r, 0:WO], Hb[:, 0:r, 0:WO], MAX)
                    t.then_inc(sem_dve, 1)
    return nc


_NC_CACHE = {}


def _get_nc(use_gpsimd=False):
    key = use_gpsimd
    if key not in _NC_CACHE:
        _NC_CACHE[key] = _build_nc(use_gpsimd=use_gpsimd)
    return _NC_CACHE[key]


def make_in_maps(noised_image):
    """f32 [32,3,512,512] -> per-core {'x': [6, 514, 1028] f16 interleaved}."""
    per = noised_image.shape[0] // N_CORES
    in_maps = []
    for c in range(N_CORES):
        shard = noised_image[c * per:(c + 1) * per].reshape(N_CH, H, W)
        shard16 = shard.astype(np.float16)
        padded = np.pad(shard16, ((0, 0), (1, 1), (1, 1)), mode='edge')
        inter = np.empty((N_STR, HP, WI), dtype=np.float16)
        inter[:, :, 0::2] = padded[0::2]
        inter[:, :, 1::2] = padded[1::2]
        in_maps.append({"x": np.ascontiguousarray(inter)})
    return in_maps


def kernel(noised_image, cover_image):
    noised_image = np.ascontiguousarray(noised_image, dtype=np.float32)
    nc = _get_nc()
    in_maps = make_in_maps(noised_image)
    res = bass_utils.run_bass_kernel_spmd(nc, in_maps, core_ids=list(range(N_CORES)))
    blurred = np.empty((N_CORES, N_CH, H, W), dtype=np.float16)
    for c, r in enumerate(res.results):
        yc = np.asarray(r["y"]).reshape(N_STR, H, W, 2)
        blurred[c, 0::2] = yc[..., 0]
        blurred[c, 1::2] = yc[..., 1]
    return (blurred.reshape(noised_image.shape).astype(np.float32), cover_image)
